# revision 26
# baseline (speedup 1.0000x reference)
"""Multi-head self-attention (b=4, n=2048, f=1024, h=16) on 8 trn2 NeuronCores.

Sharding: core c -> batch c//2, head-half c%2 (8 heads of 64 dims each).
Each core computes its 8 heads' attention and a partial output projection
(attn_slice @ Wo_rows); host sums the two partials per batch and adds bo.

v4 over v3 (scheduling rework; math identical) — ~397-401us vs 409us:
  - x8/xT loads are token-window-major and the q/k weights fhc-major, so
    the first QK needs only ~0.75MB of DMA; a dependency-free warm-up
    matmul chain bridges the ~13us DMA-startup window (HAM stays at
    K=8/8 for the whole kernel); exp starts ~21us instead of ~45us.
  - one global QK issue queue over all 256 (block, pair, j) steps keeps
    the exp stream 2 tiles ahead across every pair/block boundary; pure
    blocks run exactly at the ACT floor (997ns per [128,1024] chunk).
  - all deferred projection work (v chunks, kT/qT windows) lives in one
    deadline-ordered pending queue, drained into per-step PE slack and
    forced by group triggers right before first use.
  - pure-pair epilogues: reciprocal chain starts at the boundary but the
    PE broadcast ([na,64] e_r stationary into a freed acc bank) plus the
    normalize muls are DEFERRED to the next pair's j==2, so the PE FIFO
    never stalls on the DVE chain.  Dual pairs keep the DRAM broadcast
    (no PE ops).  s0/s1 are initialized with 3 tiny strided memsets (the
    pad/gap/spill columns only) instead of 14us of full-tile memsets.
  - the last O-projection block overlaps the final epilogue (fc 0-2 on
    spare pP slots at j==13, the rest on four distinct acc banks, fins
    alternating DVE/ACT and both DMA queues); y is fp16 (host sums
    partials in fp32).
"""

import sys

sys.path.insert(0, "/opt/trn_rl_repo")

import numpy as np
import ml_dtypes

import concourse.bass as bass
import concourse.bacc as bacc
import concourse.mybir as mybir
import concourse.tile as tile
from concourse import bass_utils

BF16 = mybir.dt.bfloat16
F32 = mybir.dt.float32
F16 = mybir.dt.float16
FP8 = mybir.dt.float8e4
NPBF16 = ml_dtypes.bfloat16
NPFP8 = ml_dtypes.float8_e4m3
DR = mybir.MatmulPerfMode.DoubleRow
MUL = mybir.AluOpType.mult
ADD = mybir.AluOpType.add

B, N, F, H, HD = 4, 2048, 1024, 16, 64
FH = 512          # features per core (8 heads)
NC_ = 8           # cores
NTOKC = N // 128  # 16 token chunks
NJ = N // 128     # 16 key chunks
NPAIR = 4         # head pairs per core
EXPFN = mybir.ActivationFunctionType.Exp
SROW = 66         # stationary cols per head
SJ = 8 * SROW     # stationary cols per key chunk (528)
WS = 32.0         # q/k weight pre-scale (escapes fp8 subnormals)


def _ap3(t, off, s1, n1, s2, n2):
    """3D AP view [128, n1 (stride s1), n2 (stride s2)] at t+off."""
    return bass.AP(tensor=t.tensor, offset=t.offset + off,
                   ap=[t.ap[0], [s1, n1], [s2, n2]])


def _emit(nc, tc, d, sorted_mode):
    consts = tc.alloc_tile_pool(name="consts", bufs=1)
    persist = tc.alloc_tile_pool(name="persist", bufs=1)

    # ---- persistent activations ----------------------------------------
    qT_sb = persist.tile([128, 4 * N], BF16)   # [feat, tok], fhc at cols fhc*N
    kT_sb = persist.tile([128, 4 * N], BF16)
    s0_sb = persist.tile([128, NJ * SJ + 64], BF16)  # [j][h][66]: [vb|1]
    s1_sb = persist.tile([128, NJ * SJ + 64], BF16)  # e^{+m} * [vb|1]
    attnT = persist.tile([128, 4 * N], BF16)   # normalized attn, [feat, tok]

    # ================= phase 1 (lead-in) =================================
    p1sb = tc.alloc_tile_pool(name="p1sb", bufs=1)
    pkt = tc.alloc_tile_pool(name="pkt", bufs=1, space="PSUM")

    bqk = consts.tile([128, 8], F32)       # bq chunks (0-3), bk (4-7)
    nc.gpsimd.dma_start(out=bqk, in_=d["bqk"])
    # warm-up source first on the DVE queue so the PE can start ASAP
    warmsrc = consts.tile([128, 512], BF16)
    nc.vector.memset(warmsrc[:], 0.5)
    # only the columns the 128-wide AV stationary reads but no v-fin
    # writes need initializing: per-head gap col 65, the 62-col spill
    # strip at each chunk boundary, and the tail pad (full memsets would
    # occupy the DVE for ~14us right when the projection fins need it)
    for s_sb in (s0_sb, s1_sb):
        nc.vector.memset(bass.AP(tensor=s_sb.tensor, offset=s_sb.offset + 65,
                                 ap=[s_sb.ap[0], [SROW, NJ * 8], [1, 1]]), 0.0)
        nc.vector.memset(bass.AP(tensor=s_sb.tensor, offset=s_sb.offset + 528,
                                 ap=[s_sb.ap[0], [SJ, NJ - 1], [1, 62]]), 0.0)
        nc.vector.memset(s_sb[:, NJ * SJ: NJ * SJ + 64], 0.0)
    # exp table warm-up (~2.7us) while DMAs run
    warm = consts.tile([128, 8], BF16)
    nc.scalar.activation(out=warm, in_=bqk, func=EXPFN, scale=0.0)
    # bc4: e_r broadcast stationaries; region r is [4,64] with row r ones
    bc4 = consts.tile([4, 256], BF16)
    nc.gpsimd.dma_start(out=bc4, in_=d["bc4"])

    wk_sb = p1sb.tile([128, 8 * FH], FP8)  # [g 4][pl 2][col 512], x32
    wq_sb = p1sb.tile([128, 8 * FH], FP8)
    x8_sb = p1sb.tile([128, 8 * N], FP8)

    def x8_win(tb):
        for fc in range(8):
            nc.sync.dma_start(
                out=x8_sb[:, fc * N + tb * 512: fc * N + (tb + 1) * 512],
                in_=d["x8"][fc * 128:(fc + 1) * 128, tb * 512:(tb + 1) * 512])

    # the sync queue carries ONLY the exp-critical lead inputs; the
    # weights are packed fhc-major so just the two 128KB fhc=0 chunks
    # precede x8 window 0 (small consts ride the gpsimd queue)
    def w_chunk(w_sb, name, fhc):
        nc.sync.dma_start(out=w_sb[:, fhc * 1024:(fhc + 1) * 1024],
                          in_=d[name][:, fhc * 1024:(fhc + 1) * 1024])
    w_chunk(wk_sb, "wkp", 0)
    w_chunk(wq_sb, "wqp", 0)
    x8_win(0)
    x8_win(1)
    for fhc in range(1, 4):
        w_chunk(wk_sb, "wkp", fhc)
        w_chunk(wq_sb, "wqp", fhc)
    x8_win(2)
    x8_win(3)
    # v-path loads on the gpsimd queue (overlaps sync-queue descriptor
    # gen); wv + xT window 0 lead so v0 can project early
    xT_sb = p1sb.tile([128, 8 * N], BF16)
    wv_sb = p1sb.tile([128, 8 * FH], BF16)

    def xT_win(tb):
        for fc in range(8):
            nc.gpsimd.dma_start(
                out=xT_sb[:, fc * N + tb * 512: fc * N + (tb + 1) * 512],
                in_=d["xT"][fc * 128:(fc + 1) * 128, tb * 512:(tb + 1) * 512])

    for fc in range(8):
        nc.gpsimd.dma_start(out=wv_sb[:, fc * FH:(fc + 1) * FH],
                            in_=d["wv"][fc * 128:(fc + 1) * 128, :])
    ep1 = consts.tile([128, NTOKC], F32)   # e^{+m}
    nc.gpsimd.dma_start(out=ep1, in_=d["ep1"])
    bvb = consts.tile([128, FH], F32)
    nc.gpsimd.dma_start(out=bvb, in_=d["bvb"])
    xT_win(0)
    mr4 = consts.tile([4, N], F32)         # select rows [m, 1-m, m, 1-m]
    nc.gpsimd.dma_start(out=mr4, in_=d["mr4"])
    mjb2 = consts.tile([128, NJ], F32)     # exp bias for block 2: mu*m_j
    nc.gpsimd.dma_start(out=mjb2, in_=d["mjb2"])
    for tb in range(1, 4):
        xT_win(tb)
    wo_sb = consts.tile([128, 4 * 1024], BF16)
    for fc in range(4):
        nc.gpsimd.dma_start(out=wo_sb[:, fc * 1024:(fc + 1) * 1024],
                            in_=d["wo"][fc * 128:(fc + 1) * 128, :])

    # HAM warm-up: one long accumulating matmul chain on the memset tile
    # (no DMA dependency, streams back-to-back) keeps the PE busy through
    # the ~13us DMA-startup window so the clock is boosted (and stays
    # boosted) when the first projections land
    warmpk = pkt.tile([128, 512], F32, tag="pp3", name="warmpk")
    for i in range(26):
        nc.tensor.matmul(warmpk[:], warmsrc[:, 0:128], warmsrc[:],
                         start=(i == 0), stop=(i == 25))

    pools = {"proj": pkt}  # phase 2 rebinds this to its own PSUM pool

    def proj_qk_ops(w_sb, fhc, win, bias_col, out_sb, tagsel, grp):
        """Micro-ops for one [128,512] window of a q/k DR-fp8 projection:
        4 paired-contraction matmuls plus the scale+bias add."""
        st = {}
        ops = []
        for g in range(4):
            def mm(g=g, st=st):
                if g == 0:
                    # "logits"-tagged windows borrow a pP rotation slot
                    # (safe when all four acc banks hold live accumulators)
                    pool = (pools.get("pP") if tagsel == "logits"
                            else None) or pools["proj"]
                    st["pk"] = pool.tile([128, 512], F32,
                                         tag=tagsel, name="pk")
                lhsT = _ap3(w_sb, fhc * 1024 + g * 256, 128, 2, 1, 128)
                rhs = _ap3(x8_sb, (2 * g) * N + win * 512, N, 2, 1, 512)
                nc.tensor.matmul(st["pk"][:], lhsT, rhs, start=(g == 0),
                                 stop=(g == 3), perf_mode=DR)
            ops.append(("pe", grp, mm))

        def fin(st=st):
            nc.vector.tensor_scalar(
                out=out_sb[:, fhc * N + win * 512: fhc * N + win * 512 + 512],
                in0=st["pk"][:], scalar1=1.0 / WS,
                scalar2=bqk[:, bias_col:bias_col + 1], op0=MUL, op1=ADD)
        ops.append(("dve", grp, fin))
        return ops

    vstg = p1sb.tile([128, FH], F32, tag="vstg")

    def v_ops(tokc, tagsel):
        st = {}
        ops = []
        grp = f"v{tokc}"
        for fc in range(8):
            def mm(fc=fc, tokc=tokc, st=st):
                if fc == 0:
                    st["pv"] = pools["proj"].tile([128, FH], F32,
                                                  tag=tagsel, name="pv")
                nc.tensor.matmul(
                    st["pv"][:],
                    xT_sb[:, fc * N + tokc * 128: fc * N + (tokc + 1) * 128],
                    wv_sb[:, fc * FH:(fc + 1) * FH],
                    start=(fc == 0), stop=(fc == 7))
            ops.append(("pe", grp, mm))

        def fin(tokc=tokc, st=st):
            nc.vector.tensor_add(out=vstg, in0=st["pv"][:], in1=bvb)
            base = tokc * SJ
            vv = vstg[:].rearrange("p (h c) -> p h c", h=8)
            for s_sb, scol in ((s0_sb, None), (s1_sb, ep1)):
                sv = s_sb[:, base:base + SJ].rearrange("p (h c) -> p h c", h=8)
                if scol is None:
                    nc.vector.tensor_copy(out=sv[:, :, 0:64], in_=vv)
                    nc.vector.memset(sv[:, :, 64:65], 1.0)
                else:
                    nc.vector.tensor_scalar_mul(
                        out=sv[:, :, 0:64], in0=vv,
                        scalar1=scol[:, tokc:tokc + 1])
                    colb = bass.AP(
                        tensor=scol.tensor,
                        offset=scol[:, tokc:tokc + 1].offset,
                        ap=[scol[:, tokc:tokc + 1].ap[0], [0, 8], [1, 1]])
                    nc.vector.tensor_copy(out=sv[:, :, 64:65], in_=colb)
        ops.append(("dve", grp, fin))
        return ops

    # ---- up-front minimum: kT c0 w0, qT c0 w0, v0 ----------------------
    iblk_order = [0, 2, 1, 3] if sorted_mode else [0, 1, 2, 3]
    b0 = iblk_order[0]
    for _, _, op in proj_qk_ops(wk_sb, 0, 0, 4, kT_sb, "pp0", "k0w0"):
        op()
    for _, _, op in proj_qk_ops(wq_sb, 0, b0, 0, qT_sb, "pp2", "q0b"):
        op()

    # ---- deferred work, deadline-ordered --------------------------------
    # deadline unit = one exp step from the start of attention
    items = []  # (deadline, ops)
    for j in range(1, NTOKC):
        items.append((j, v_ops(j, "acc1" if j % 2 else "acc3")))
    for c in range(NPAIR):
        for w in range(4):
            if c == 0 and w == 0:
                continue
            dl = (16 * c + 4 * w - 2) if c else (4 * w - 2)
            items.append((dl, proj_qk_ops(wk_sb, c, w, 4 + c, kT_sb,
                                          "acc1" if w % 2 else "acc3",
                                          f"k{c}w{w}")))
    for bi, iblk in enumerate(iblk_order):
        for c in range(NPAIR):
            if bi == 0 and c == 0:
                continue
            # drain each q window during the PREVIOUS block, where the
            # acc1/acc3 banks are free; the last block's windows can only
            # drain inside the dual block (all acc banks live), so they
            # ride the pP "logits" rotation instead
            if bi == 0:
                dl, tag = 16 * c - 2, ("acc1", "acc3")[c % 2]
            elif bi == len(iblk_order) - 1:
                dl, tag = 64 * (bi - 1) + 16 * c + 14, "logits"
            else:
                dl, tag = 64 * (bi - 1) + 16 * c + 14, ("acc1", "acc3")[c % 2]
            items.append((dl, proj_qk_ops(wq_sb, c, iblk, c, qT_sb,
                                          tag, f"q{c}b{iblk}")))
    if not sorted_mode:
        # correctness-only fallback: run everything up front
        for _, ops in sorted(items, key=lambda x: x[0]):
            for _, _, op in ops:
                op()
        items = []
    pending = []
    for _, ops in sorted(items, key=lambda x: x[0]):
        pending.extend(ops)

    pkt.release()
    if not sorted_mode:
        p1sb.release()

    def drain(npe):
        done = 0
        while pending and done < npe:
            kind, _, op = pending.pop(0)
            op()
            if kind == "pe":
                done += 1
        while pending and pending[0][0] == "dve":
            pending.pop(0)[2]()

    def drain_group(grp):
        """Pop (in order) until no ops of group `grp` remain."""
        while any(g == grp for _, g, _ in pending):
            pending.pop(0)[2]()

    # ================= phase 2: attention ================================
    with tc.tile_pool(name="pP", bufs=2, space="PSUM") as pP, \
         tc.tile_pool(name="pacc", bufs=1, space="PSUM") as pacc, \
         tc.tile_pool(name="sexp", bufs=4) as sexp, \
         tc.tile_pool(name="episb", bufs=1) as episb, \
         tc.tile_pool(name="rblp", bufs=1) as rblp, \
         tc.tile_pool(name="epidr", bufs=2, space="DRAM") as epidr, \
         tc.tile_pool(name="osb", bufs=4) as osb:

        pools["proj"] = pacc
        pools["pP"] = pP

        def o_unit_ops(tokc, half, tag, fin_act, pool=None):
            """One O-projection unit: 4 accumulating matmuls + fin."""
            st = {}
            ops = []
            for fc in range(4):
                def mm(fc=fc, st=st):
                    if fc == 0:
                        st["po"] = (pool or pacc).tile([128, 512], F32,
                                                       tag=tag, name="po")
                    nc.tensor.matmul(
                        st["po"][:],
                        attnT[:, fc * N + tokc * 128: fc * N + (tokc + 1) * 128],
                        wo_sb[:, fc * 1024 + half * 512: fc * 1024 + half * 512 + 512],
                        start=(fc == 0), stop=(fc == 3))
                ops.append(("pe", f"o{tokc}", mm))

            def fin(st=st):
                ot = osb.tile([128, 512], F16, tag="ot", name="ot")
                if fin_act:
                    nc.scalar.activation(
                        out=ot, in_=st["po"][:],
                        func=mybir.ActivationFunctionType.Copy)
                else:
                    nc.vector.tensor_copy(out=ot, in_=st["po"][:])
                nc.sync.dma_start(
                    out=d["y"][tokc * 128:(tokc + 1) * 128,
                               half * 512:(half + 1) * 512],
                    in_=ot)
            ops.append(("dve", f"o{tokc}", fin))
            return ops

        def o_ops_for_iblk(ib, tags, fin_act=False):
            ops = []
            from itertools import cycle
            tagc = cycle(tags)
            for tokc in range(ib * 4, ib * 4 + 4):
                for half in range(2):
                    ops.extend(o_unit_ops(tokc, half, next(tagc), fin_act))
            return ops

        # global QK issue stream: 2 exp-steps ahead across all boundaries
        pairs_seq = [(iblk, pair) for iblk in iblk_order
                     for pair in range(NPAIR)]
        Pready = {}

        def make_qk(iblk, pair, j):
            def qk():
                if j % 4 == 0:
                    drain_group(f"k{pair}w{j // 4}")
                if j == 0 and not (iblk == iblk_order[0] and pair == 0):
                    drain_group(f"q{pair}b{iblk}")
                P = pP.tile([128, 1024], F32, tag="logits")
                for hl, tp in ((0, 0), (1, 64)):
                    nc.tensor.matmul(
                        P[:, hl * 512:(hl + 1) * 512],
                        kT_sb[tp:tp + 64, pair * N + j * 128: pair * N + (j + 1) * 128],
                        qT_sb[tp:tp + 64, pair * N + iblk * 512: pair * N + (iblk + 1) * 512],
                        start=True, stop=True, tile_position=(tp, 0))
                return P
            return ((iblk, pair, j), qk)

        qkq = [make_qk(ib, pr, j) for ib, pr in pairs_seq for j in range(NJ)]

        def issue_qk():
            key, fn = qkq.pop(0)
            Pready[key] = fn()

        def epilogue(iblk, pair, accs, active, stats, last_pair):
            """attnT[:, pair*N + iblk*512 : +512] = normalized attention.

            Pure pairs: emits the acc copies + PSUM-direct denominator
            gather + reciprocal now; returns a closure with the PE
            broadcast matmuls + normalize muls, deferred into the next
            pair's stream so the PE never waits on the DVE chain.
            Dual pairs: fully inline via the DRAM broadcast (no PE ops).
            """
            dual = len(active) == 4
            dstc = pair * N + iblk * 512
            rin = episb.tile([4, 512], F32, tag="rin")
            rows = {v: k for k, v in enumerate(active)}
            asb = {}
            for v in active:
                t = episb.tile([65, 512], F32, tag=f"asb{v}", name=f"asb{v}")
                if last_pair:
                    nc.scalar.activation(
                        out=t, in_=accs[v][0:65, :],
                        func=mybir.ActivationFunctionType.Copy)
                else:
                    nc.vector.tensor_copy(out=t, in_=accs[v][0:65, :])
                asb[v] = t
                if not dual:
                    nc.sync.dma_start(out=rin[rows[v]:rows[v] + 1, :],
                                      in_=t[64:65, :])

            if dual:
                rinA = episb.tile([2, 512], F32, tag="rinA")
                rinB = episb.tile([2, 512], F32, tag="rinB")
                rtile = {v: ((rinA, rinB)[v // 2], v % 2) for v in active}
                for v in active:
                    t, r = rtile[v]
                    nc.sync.dma_start(out=t[r:r + 1, :],
                                      in_=asb[v][64:65, :])
                nc.vector.reciprocal_approx_fast(out=rinA[:], in_=rinA[:])
                nc.vector.reciprocal_approx_fast(out=rinB[:], in_=rinB[:])
                ib = iblk * 512
                nc.vector.tensor_mul(out=rinA[:], in0=rinA[:],
                                     in1=mr4[0:2, ib:ib + 512])
                nc.vector.tensor_mul(out=rinB[:], in0=rinB[:],
                                     in1=mr4[0:2, ib:ib + 512])
                stg2 = epidr.tile([4, 512], F32, tag="stg2")
                for k, v in enumerate(active):
                    t, r = rtile[v]
                    nc.sync.dma_start(out=stg2[k:k + 1, :], in_=t[r:r + 1, :])
                rball = rblp.tile([64, 4 * 512], F32, tag="rball")
                nc.sync.dma_start(
                    out=rball[:, 0:4 * 512],
                    in_=bass.AP(tensor=stg2.tensor, offset=stg2.offset,
                                ap=[[0, 64], [512, 4], [1, 512]]))
                rb = {v: rball[:, k * 512:(k + 1) * 512]
                      for k, v in enumerate(active)}
                for hl in range(2):
                    v1, v0 = 2 * hl, 2 * hl + 1
                    t1 = episb.tile([64, 512], F32, tag="ept1")
                    t2 = episb.tile([64, 512], F32, tag="ept2")
                    nc.vector.tensor_mul(out=t1, in0=asb[v1][0:64, :],
                                         in1=rb[v1])
                    nc.vector.tensor_mul(out=t2, in0=asb[v0][0:64, :],
                                         in1=rb[v0])
                    if hl == 0:
                        nc.vector.tensor_add(
                            out=attnT[0:64, dstc:dstc + 512], in0=t1, in1=t2)
                    else:
                        t3 = episb.tile([64, 512], BF16, tag="ept3")
                        nc.vector.tensor_add(out=t3, in0=t1, in1=t2)
                        nc.sync.dma_start(
                            out=attnT[64:128, dstc:dstc + 512], in_=t3)
                return None

            rin16 = episb.tile([4, 512], BF16, tag="rin16")
            nc.vector.reciprocal_approx_fast(out=rin[0:2, :], in_=rin[0:2, :])
            nc.vector.tensor_copy(out=rin16[0:2, :], in_=rin[0:2, :])
            free = [v for v in (0, 1, 2, 3) if v not in asb]

            def post():
                rb = {}
                for v in active:
                    r = rows[v]
                    t = pacc.tile([64, 512], F32, tag=f"acc{free[r]}",
                                  name="rb")
                    nc.tensor.matmul(t[:], bc4[0:2, r * 64:r * 64 + 64],
                                     rin16[0:2, :], start=True, stop=True)
                    rb[v] = t
                for hl in range(2):
                    v1, v0 = 2 * hl, 2 * hl + 1
                    vv = v1 if v1 in asb else v0
                    if hl == 0:
                        nc.vector.tensor_mul(
                            out=attnT[0:64, dstc:dstc + 512],
                            in0=asb[vv][0:64, :], in1=rb[vv][0:64, :])
                    else:
                        t3 = episb.tile([64, 512], BF16, tag="ept3")
                        nc.vector.tensor_mul(out=t3, in0=asb[vv][0:64, :],
                                             in1=rb[vv][0:64, :])
                        nc.sync.dma_start(
                            out=attnT[64:128, dstc:dstc + 512], in_=t3)
            return post

        # ---- main loop --------------------------------------------------
        post_pending = []
        o3a = []
        first_pair = True
        for iblk, pair in pairs_seq:
            if sorted_mode and iblk == 0:
                active, stats = [0, 2], {0: s1_sb, 2: s1_sb}
            elif sorted_mode and iblk == 2:
                active, stats = [0, 2], {0: s0_sb, 2: s0_sb}
            elif sorted_mode and iblk == 3:
                active, stats = [1, 3], {1: s0_sb, 3: s0_sb}
            else:
                active = [0, 1, 2, 3]
                stats = {0: s1_sb, 1: s0_sb, 2: s1_sb, 3: s0_sb}
            dual = len(active) == 4
            first_blk_p0 = (iblk == iblk_order[0] and pair == 0)
            last_pair = (iblk == iblk_order[-1] and pair == NPAIR - 1)
            ndrain = 6 if iblk == iblk_order[0] else 2

            if dual and post_pending:
                # a dual pair reuses every acc bank; finish the deferred
                # broadcast before our accumulators claim the tags
                for f in post_pending:
                    f()
                post_pending = []
            accs = {
                v: pacc.tile([128, 512], F32, tag=f"acc{v}", name=f"acc{v}")
                for v in active
            }
            while len(Pready) < 2 and qkq:
                issue_qk()
            if first_pair:
                # v0 races the first two exp steps; AV j0 waits on its fin
                for _, _, op in v_ops(0, "acc1"):
                    op()
                first_pair = False
            for j in range(NJ):
                if j == 2 and post_pending:
                    for f in post_pending:
                        f()
                    post_pending = []
                if sorted_mode and first_blk_p0 and j >= 1:
                    drain_group(f"v{j}")
                S = sexp.tile([128, 1024], BF16, tag="etil")
                ebias = (mjb2[:, j:j + 1]
                         if (sorted_mode and iblk == 2) else 0.0)
                nc.scalar.activation(out=S[:],
                                     in_=Pready.pop((iblk, pair, j)),
                                     func=EXPFN, scale=1.0 / 32.0,
                                     bias=ebias)
                if qkq:
                    issue_qk()
                for hl in range(2):
                    hcore = 2 * pair + hl
                    soff = j * SJ + hcore * SROW
                    rhs = S[:, hl * 512:(hl + 1) * 512]
                    for v in (2 * hl, 2 * hl + 1):
                        if v not in accs:
                            continue
                        # 128-wide stationary read (cols 65+ produce
                        # ignored partitions) keeps FWL enabled
                        nc.tensor.matmul(
                            accs[v][:], stats[v][:, soff:soff + 128], rhs,
                            start=(j == 0), stop=(j == NJ - 1))
                # j >= 3 keeps popped O-projection reads ordered after the
                # previous pair's deferred attnT writes (flushed at j == 2)
                if not dual and j >= 3:
                    drain(ndrain)
                if last_pair and j == 13:
                    # the QK stream is exhausted; fill the last steps' PE
                    # slack with the final O-block's first 3 fc planes
                    for k in range(2):
                        big = pP.tile([128, 1024], F32, tag="logits",
                                      name="po2")
                        for h2 in range(2):
                            po = big[:, h2 * 512:(h2 + 1) * 512]
                            for fc in range(3):
                                nc.tensor.matmul(
                                    po,
                                    attnT[:, fc * N + (12 + k) * 128: fc * N + (13 + k) * 128],
                                    wo_sb[:, fc * 1024 + h2 * 512: fc * 1024 + h2 * 512 + 512],
                                    start=(fc == 0), stop=False)
                            o3a.append((12 + k, h2, po))

            if last_pair:
                while pending:
                    pending.pop(0)[2]()
                post = epilogue(iblk, pair, accs, active, stats, True)
                if post:
                    post()
                fins = []
                for tokc, half, po in o3a:
                    nc.tensor.matmul(
                        po,
                        attnT[:, 3 * N + tokc * 128: 3 * N + (tokc + 1) * 128],
                        wo_sb[:, 3 * 1024 + half * 512: 3 * 1024 + half * 512 + 512],
                        start=False, stop=True)
                    fins.append((tokc, half, po))
                # remaining 4 units on 4 distinct acc banks so the PE can
                # stream all 16 matmuls without waiting on a fin
                for k, (tokc, half) in enumerate(
                        ((14, 0), (14, 1), (15, 0), (15, 1))):
                    st = {}
                    for fc in range(4):
                        if fc == 0:
                            st["po"] = pacc.tile([128, 512], F32,
                                                 tag=f"acc{k}", name="po")
                        nc.tensor.matmul(
                            st["po"][:],
                            attnT[:, fc * N + tokc * 128: fc * N + (tokc + 1) * 128],
                            wo_sb[:, fc * 1024 + half * 512: fc * 1024 + half * 512 + 512],
                            start=(fc == 0), stop=(fc == 3))
                    fins.append((tokc, half, st["po"][:]))
                for k, (tokc, half, po) in enumerate(fins):
                    ot = osb.tile([128, 512], F16, tag="ot", name="ot")
                    if k % 2:
                        nc.scalar.activation(
                            out=ot, in_=po,
                            func=mybir.ActivationFunctionType.Copy)
                    else:
                        nc.vector.tensor_copy(out=ot, in_=po)
                    q = nc.sync if k % 2 else nc.gpsimd
                    q.dma_start(
                        out=d["y"][tokc * 128:(tokc + 1) * 128,
                                   half * 512:(half + 1) * 512],
                        in_=ot)
            else:
                post = epilogue(iblk, pair, accs, active, stats, False)
                if post:
                    post_pending.append(post)
                if pair == NPAIR - 1:
                    # this block's O-projection drains into later slack;
                    # tags must be the FREE acc banks of the block where
                    # the ops drain: o(first) drains in block 2 (free
                    # {1,3}), the rest drain in block 3 (free {0,2})
                    tags = (("acc1", "acc3") if iblk == iblk_order[0]
                            else ("acc0", "acc2"))
                    pending.extend(o_ops_for_iblk(iblk, tags))

    if sorted_mode:
        p1sb.release()
    persist.release()
    consts.release()


_CACHE = {}


def build_program(variant="sorted"):
    if variant in _CACHE:
        return _CACHE[variant]
    nc = bacc.Bacc("TRN2", target_bir_lowering=False, debug=False)
    d = {}
    d["xT"] = nc.dram_tensor("xT", (F, N), BF16, kind="ExternalInput").ap()
    d["x8"] = nc.dram_tensor("x8", (F, N), FP8, kind="ExternalInput").ap()
    d["wqp"] = nc.dram_tensor("wqp", (128, 8 * FH), FP8, kind="ExternalInput").ap()
    d["wkp"] = nc.dram_tensor("wkp", (128, 8 * FH), FP8, kind="ExternalInput").ap()
    d["wv"] = nc.dram_tensor("wv", (F, FH), BF16, kind="ExternalInput").ap()
    d["wo"] = nc.dram_tensor("wo", (FH, F), BF16, kind="ExternalInput").ap()
    d["bqk"] = nc.dram_tensor("bqk", (128, 8), F32, kind="ExternalInput").ap()
    d["bvb"] = nc.dram_tensor("bvb", (128, FH), F32, kind="ExternalInput").ap()
    d["ep1"] = nc.dram_tensor("ep1", (128, NTOKC), F32, kind="ExternalInput").ap()
    d["mjb2"] = nc.dram_tensor("mjb2", (128, NJ), F32, kind="ExternalInput").ap()
    d["mr4"] = nc.dram_tensor("mr4", (4, N), F32, kind="ExternalInput").ap()
    d["bc4"] = nc.dram_tensor("bc4", (4, 256), BF16, kind="ExternalInput").ap()
    d["y"] = nc.dram_tensor("y", (N, F), F16, kind="ExternalOutput").ap()
    with tile.TileContext(nc) as tc:
        _emit(nc, tc, d, sorted_mode=(variant == "sorted"))
    nc.compile()
    _CACHE[variant] = nc
    return nc


def _wpack_qk(w):
    """W [1024, 512] fp32 -> [128, 4096] fp8 x32: [part][fhc 4][g 4][pl 2][128]."""
    a = (w * WS).reshape(4, 2, 128, 4, 128)      # [g, pl, part, fhc, col]
    a = a.transpose(2, 3, 0, 1, 4).reshape(128, 8 * FH)
    return a.astype(NPFP8)


def _perm_blocks(m):
    """Permutation putting tokens into blocks: 0 pure-1, 1 mixed, 2 pure
    (mu = c1 > 1024), 3 pure-0. Returns perm, mu."""
    ones = np.flatnonzero(m > 0.5)
    zeros = np.flatnonzero(m <= 0.5)
    c1 = len(ones)
    if c1 > 1024:
        mu = 1.0
        perm = np.concatenate([
            ones[0:512], ones[1024:], zeros[0:1536 - c1],
            ones[512:1024], zeros[1536 - c1:]])
    else:
        mu = 0.0
        perm = np.concatenate([
            ones[0:512], ones[512:], zeros[0:1024 - c1],
            zeros[1024 - c1:1536 - c1], zeros[1536 - c1:]])
    return perm, mu


def make_in_maps(x, inputs_mask, Wq, bq, Wk, bk, Wv, bv, Wo, bo,
                 sorted_mode=True):
    in_maps = []
    m_all = inputs_mask.astype(np.float32)
    perms, mus = [], []
    for b in range(B):
        if sorted_mode:
            p, mu = _perm_blocks(m_all[b])
        else:
            p, mu = np.arange(N), 0.0
        perms.append(p)
        mus.append(mu)
    for c in range(NC_):
        b, hh = c // 2, c % 2
        cs = slice(hh * FH, (hh + 1) * FH)
        m = m_all[b][perms[b]]
        xb = x[b][perms[b]]
        xT = np.ascontiguousarray(xb.T)
        im = {
            "xT": xT.astype(NPBF16),
            "x8": xT.astype(NPFP8),
            "wqp": _wpack_qk(Wq[:, cs]),
            "wkp": _wpack_qk(Wk[:, cs]),
            "wv": Wv[:, cs].astype(NPBF16),
            "wo": np.ascontiguousarray(Wo[cs, :]).astype(NPBF16),
            "bqk": np.stack(
                [bq[cs].reshape(4, 128), bk[cs].reshape(4, 128)], axis=0
            ).reshape(8, 128).T.astype(np.float32).copy(),
            "bvb": np.broadcast_to(bv[cs], (128, FH)).astype(np.float32).copy(),
            "ep1": np.exp(m).reshape(NTOKC, 128).T.astype(np.float32).copy(),
            "mjb2": (mus[b] * m).reshape(NJ, 128).T.astype(np.float32).copy(),
            "mr4": np.stack([m, 1.0 - m, m, 1.0 - m]).astype(np.float32).copy(),
            "bc4": np.kron(np.eye(4), np.ones((1, 64))).astype(NPBF16),
        }
        in_maps.append(im)
    return in_maps, perms


def kernel(x, inputs_mask, Wq, bq, Wk, bk, Wv, bv, Wo, bo):
    x = np.asarray(x, dtype=np.float32)
    inputs_mask = np.asarray(inputs_mask)
    Wq, bq = np.asarray(Wq, np.float32), np.asarray(bq, np.float32)
    Wk, bk = np.asarray(Wk, np.float32), np.asarray(bk, np.float32)
    Wv, bv = np.asarray(Wv, np.float32), np.asarray(bv, np.float32)
    Wo, bo = np.asarray(Wo, np.float32), np.asarray(bo, np.float32)

    c1 = inputs_mask.astype(np.int64).sum(axis=1)
    sorted_mode = bool(np.all((c1 >= 512) & (c1 <= 3 * 512)))
    nc = build_program("sorted" if sorted_mode else "dual")
    in_maps, perms = make_in_maps(
        x, inputs_mask, Wq, bq, Wk, bk, Wv, bv, Wo, bo, sorted_mode=sorted_mode)
    res = bass_utils.run_bass_kernel_spmd(nc, in_maps, core_ids=list(range(NC_)))
    out = np.empty((B, N, F), dtype=np.float32)
    for b in range(B):
        out[b][perms[b]] = (
            res.results[2 * b]["y"].astype(np.float32)
            + res.results[2 * b + 1]["y"].astype(np.float32) + bo
        )
    return out
